# revision 4
# baseline (speedup 1.0000x reference)
"""Trainium2 Bass kernel for nn_CustomAttentionLayer (GNN message passing).

Math reformulation (exact to fp32 rounding):
  gate depends only on the source node: g[v] = x[v]@w_gate + b_gate
  egv = exp(g)  (no max-shift needed; |g| <~ 3)
  T = C @ [egv*x, egv]  where C[n,v] = edge multiplicity (row=n, col=v)
  S = T[:, :128] / (T[:, 128] + 1e-16);  a = T[:, 128] / (T[:, 128] + 1e-16)
  out = S @ (W_out@W_lin).T + a*(W_out@b_lin) + b_out

Distribution: destination-sharded over 8 cores (10 dest blocks of 128 nodes
per core, 79 blocks total cover 10112 >= 10000 padded nodes). Host buckets
edges by (dest block, source block); on device each bucket's count matrix
C[s, j] is built with bf16 one-hot matmuls contracted over edge tiles, then
T accumulates C^T @ Y_b in PSUM. No per-edge DMA anywhere.
"""
import numpy as np
import ml_dtypes

import concourse.bass as bass
import concourse.tile as tile
from concourse import bacc, mybir
from concourse.bass_utils import run_bass_kernel_spmd
from concourse.masks import make_identity

F32 = mybir.dt.float32
BF16 = mybir.dt.bfloat16

N_CORES = 8
N = 10000
D = 128
P = 128
NB = 79          # source blocks of 128 (79*128 = 10112)
NSB = 10         # dest blocks per core
NPAD = NB * P    # 10112
SENT = 200.0     # sentinel local index (exact in bf16, never matches 0..127)
EPS = 1e-16


def _host_prep(x, edge_index, W_lin, b_lin, W_gate, b_gate, W_out, b_out):
    row = np.asarray(edge_index[0], dtype=np.int64)
    col = np.asarray(edge_index[1], dtype=np.int64)
    E = row.shape[0]

    s_glob = row >> 7          # global dest block, 0..78
    b_glob = col >> 7          # source block, 0..78
    key = s_glob * NB + b_glob
    order = np.argsort(key, kind="stable")
    key_sorted = key[order]
    row_sorted = row[order]
    col_sorted = col[order]
    # bucket boundaries
    starts = np.searchsorted(key_sorted, np.arange(NB * NB))
    ends = np.searchsorted(key_sorted, np.arange(NB * NB) + 1)
    cnt = (ends - starts).reshape(NB, NB)  # [dest block s, src block b]

    # static tile counts per (slot k, src b): max over cores (uniform schedule)
    tpb = np.ones((NSB, NB), dtype=np.int64)
    for k in range(NSB):
        for c in range(N_CORES):
            s = 10 * c + k
            if s < NB:
                need = (cnt[s] + P - 1) // P
                tpb[k] = np.maximum(tpb[k], need)
    tiles_k = tpb.sum(axis=1)          # tiles per slot
    TT = int(tiles_k.sum())            # total tile slots

    # per-core metadata: local col/row indices per tile [128, TT] bf16
    metas = []
    for c in range(N_CORES):
        mc = np.full((TT, P), SENT, dtype=np.float32)
        mr = np.full((TT, P), SENT, dtype=np.float32)
        ti = 0
        for k in range(NSB):
            s = 10 * c + k
            for b in range(NB):
                nt = int(tpb[k, b])
                if s < NB:
                    a0, a1 = starts[s * NB + b], ends[s * NB + b]
                    n = a1 - a0
                    assert n <= nt * P, "bucket overflow vs static schedule"
                    cl = (col_sorted[a0:a1] - (b << 7)).astype(np.float32)
                    rl = (row_sorted[a0:a1] - (s << 7)).astype(np.float32)
                    buf_c = np.full(nt * P, SENT, dtype=np.float32)
                    buf_r = np.full(nt * P, SENT, dtype=np.float32)
                    buf_c[:n] = cl
                    buf_r[:n] = rl
                    mc[ti : ti + nt] = buf_c.reshape(nt, P)
                    mr[ti : ti + nt] = buf_r.reshape(nt, P)
                ti += nt
        metas.append(
            (
                np.ascontiguousarray(mc.T).astype(ml_dtypes.bfloat16),
                np.ascontiguousarray(mr.T).astype(ml_dtypes.bfloat16),
            )
        )

    x = np.asarray(x, dtype=np.float32)
    x_pad = np.zeros((NPAD, D), dtype=np.float32)
    x_pad[:N] = x

    W_lin = np.asarray(W_lin, np.float32)
    b_lin = np.asarray(b_lin, np.float32)
    W_gate = np.asarray(W_gate, np.float32)
    b_gate = np.asarray(b_gate, np.float32)
    W_out = np.asarray(W_out, np.float32)
    b_out = np.asarray(b_out, np.float32)

    wgate_rep = np.ascontiguousarray(np.broadcast_to(W_gate[0], (P, D))).astype(
        np.float32
    )
    iota_bf = np.ascontiguousarray(
        np.broadcast_to(np.arange(P, dtype=np.float32), (P, P))
    ).astype(ml_dtypes.bfloat16)
    wct = np.ascontiguousarray((W_out @ W_lin).T).astype(np.float32)  # [i, o]
    u = W_out @ b_lin
    urep = np.ascontiguousarray(np.broadcast_to(u, (P, P))).astype(np.float32)
    brep = np.ascontiguousarray(np.broadcast_to(b_out, (P, P))).astype(np.float32)

    consts = dict(
        x_pad=x_pad,
        wgate_rep=wgate_rep,
        iota_bf=iota_bf,
        wct=wct,
        urep=urep,
        brep=brep,
    )
    return metas, consts, tpb, TT, float(b_gate[0])


def _build_program(tpb, TT, bgate_scalar):
    nc = bacc.Bacc(
        "TRN2",
        target_bir_lowering=False,
        debug=False,
        enable_asserts=True,
        num_devices=N_CORES,
    )

    x_ap = nc.dram_tensor("x_pad", [NPAD, D], F32, kind="ExternalInput").ap()
    mc_ap = nc.dram_tensor("meta_cols", [P, TT], BF16, kind="ExternalInput").ap()
    mr_ap = nc.dram_tensor("meta_rows", [P, TT], BF16, kind="ExternalInput").ap()
    wg_ap = nc.dram_tensor("wgate_rep", [P, D], F32, kind="ExternalInput").ap()
    io_ap = nc.dram_tensor("iota_bf", [P, P], BF16, kind="ExternalInput").ap()
    wc_ap = nc.dram_tensor("wct", [P, P], F32, kind="ExternalInput").ap()
    ur_ap = nc.dram_tensor("urep", [P, P], F32, kind="ExternalInput").ap()
    br_ap = nc.dram_tensor("brep", [P, P], F32, kind="ExternalInput").ap()
    out_ap = nc.dram_tensor("out", [NSB * P, D], F32, kind="ExternalOutput").ap()

    tiles_k = tpb.sum(axis=1)

    with tile.TileContext(nc) as tc:
        with (
            tc.tile_pool(name="const", bufs=1) as cpool,
            tc.tile_pool(name="ybuf", bufs=1) as ybpool,
            tc.tile_pool(name="p1", bufs=3) as p1,
            tc.tile_pool(name="meta", bufs=2) as mpool,
            tc.tile_pool(name="oh", bufs=8) as ohpool,
            tc.tile_pool(name="cs", bufs=4) as cspool,
            tc.tile_pool(name="fin", bufs=2) as fpool,
            tc.tile_pool(name="cps", bufs=2, space="PSUM") as cps,
            tc.tile_pool(name="tps", bufs=2, space="PSUM") as tps,
            tc.tile_pool(name="p3ps", bufs=2, space="PSUM") as p3ps,
        ):
            wgate_t = cpool.tile([P, D], F32)
            nc.sync.dma_start(wgate_t[:], wg_ap[:])
            iota_t = cpool.tile([P, P], BF16)
            nc.sync.dma_start(iota_t[:], io_ap[:])
            wct_t = cpool.tile([P, P], F32)
            nc.sync.dma_start(wct_t[:], wc_ap[:])
            urep_t = cpool.tile([P, P], F32)
            nc.sync.dma_start(urep_t[:], ur_ap[:])
            brep_t = cpool.tile([P, P], F32)
            nc.sync.dma_start(brep_t[:], br_ap[:])
            ident_t = cpool.tile([P, P], F32)
            make_identity(nc, ident_t[:])
            bgate_t = cpool.tile([P, 1], F32)
            nc.vector.memset(bgate_t[:], bgate_scalar)

            # ---- phase 1: Y[:, b, :] = egv * [x_b | 1]  (SBUF resident) ----
            ybuf = ybpool.tile([P, NB, D + 1], F32)
            for b in range(NB):
                xt = p1.tile([P, D], F32, tag="xt")
                nc.sync.dma_start(xt[:], x_ap[b * P : (b + 1) * P, :])
                prod = p1.tile([P, D], F32, tag="prod")
                nc.vector.tensor_tensor(
                    out=prod[:], in0=xt[:], in1=wgate_t[:], op=mybir.AluOpType.mult
                )
                gt = p1.tile([P, 1], F32, tag="gt")
                nc.vector.reduce_sum(gt[:], prod[:], axis=mybir.AxisListType.X)
                egt = p1.tile([P, 1], F32, tag="egt")
                nc.scalar.activation(
                    egt[:], gt[:], mybir.ActivationFunctionType.Exp,
                    bias=bgate_t[:, 0:1],
                )
                nc.vector.tensor_scalar_mul(ybuf[:, b, 0:D], xt[:], egt[:])
                nc.scalar.copy(ybuf[:, b, D : D + 1], egt[:])

            # ---- phase 2 + 3 per dest slot ----
            moff = 0
            for k in range(NSB):
                ntk = int(tiles_k[k])
                mct = mpool.tile([P, ntk], BF16, tag="mct")
                nc.sync.dma_start(mct[:], mc_ap[:, moff : moff + ntk])
                mrt = mpool.tile([P, ntk], BF16, tag="mrt")
                nc.sync.dma_start(mrt[:], mr_ap[:, moff : moff + ntk])

                t_ps = tps.tile([P, D + 1], F32)
                ti = 0
                for b in range(NB):
                    nt = int(tpb[k, b])
                    c_ps = cps.tile([P, P], F32, tag="c_ps")
                    for t in range(nt):
                        g_oh = ohpool.tile([P, P], BF16, tag="g_oh")
                        nc.vector.tensor_tensor(
                            out=g_oh[:],
                            in0=mct[:, ti + t : ti + t + 1].to_broadcast([P, P]),
                            in1=iota_t[:],
                            op=mybir.AluOpType.is_equal,
                        )
                        o_oh = ohpool.tile([P, P], BF16, tag="o_oh")
                        nc.vector.tensor_tensor(
                            out=o_oh[:],
                            in0=mrt[:, ti + t : ti + t + 1].to_broadcast([P, P]),
                            in1=iota_t[:],
                            op=mybir.AluOpType.is_equal,
                        )
                        nc.tensor.matmul(
                            c_ps[:], lhsT=g_oh[:], rhs=o_oh[:],
                            start=(t == 0), stop=(t == nt - 1),
                        )
                    ti += nt
                    cs_t = cspool.tile([P, P], F32, tag="cs_t")
                    nc.scalar.copy(cs_t[:], c_ps[:])
                    nc.tensor.matmul(
                        t_ps[:], lhsT=cs_t[:], rhs=ybuf[:, b, :],
                        start=(b == 0), stop=(b == NB - 1),
                    )
                moff += ntk

                # ---- phase 3 for this slot ----
                ts_t = fpool.tile([P, D + 1], F32, tag="ts_t")
                nc.vector.tensor_copy(ts_t[:], t_ps[:])
                den_t = fpool.tile([P, 1], F32, tag="den_t")
                nc.vector.tensor_scalar_add(den_t[:], ts_t[:, D : D + 1], EPS)
                rec_t = fpool.tile([P, 1], F32, tag="rec_t")
                nc.vector.reciprocal(rec_t[:], den_t[:])
                tt_ps = p3ps.tile([P, P], F32, tag="tt_ps")
                nc.tensor.transpose(tt_ps[:], ts_t[:, 0:D], ident_t[:])
                st_t = fpool.tile([P, P], F32, tag="st_t")
                nc.vector.tensor_copy(st_t[:], tt_ps[:])
                m_ps = p3ps.tile([P, P], F32, tag="m_ps")
                nc.tensor.matmul(m_ps[:], lhsT=st_t[:], rhs=wct_t[:],
                                 start=True, stop=True)
                mn_t = fpool.tile([P, P], F32, tag="mn_t")
                nc.vector.tensor_scalar_mul(mn_t[:], m_ps[:], rec_t[:])
                a_t = fpool.tile([P, 1], F32, tag="a_t")
                nc.vector.tensor_scalar_mul(a_t[:], ts_t[:, D : D + 1], rec_t[:])
                au_t = fpool.tile([P, P], F32, tag="au_t")
                nc.vector.tensor_scalar_mul(au_t[:], urep_t[:], a_t[:])
                o1_t = fpool.tile([P, P], F32, tag="o1_t")
                nc.vector.tensor_add(o1_t[:], mn_t[:], au_t[:])
                o2_t = fpool.tile([P, P], F32, tag="o2_t")
                nc.vector.tensor_add(o2_t[:], o1_t[:], brep_t[:])
                nc.sync.dma_start(out_ap[k * P : (k + 1) * P, :], o2_t[:])

    nc.compile()
    return nc


def _run(inputs, trace=False):
    metas, consts, tpb, TT, bgate_scalar = _host_prep(
        inputs["x"], inputs["edge_index"], inputs["W_lin"], inputs["b_lin"],
        inputs["W_gate"], inputs["b_gate"], inputs["W_out"], inputs["b_out"],
    )
    nc = _build_program(tpb, TT, bgate_scalar)
    in_maps = []
    for c in range(N_CORES):
        mc, mr = metas[c]
        m = dict(consts)
        m["meta_cols"] = mc
        m["meta_rows"] = mr
        in_maps.append(m)
    res = run_bass_kernel_spmd(
        nc, in_maps, core_ids=list(range(N_CORES)), trace=trace
    )
    parts = [res.results[c]["out"] for c in range(N_CORES)]
    full = np.concatenate(parts, axis=0)[:N]
    return np.ascontiguousarray(full, dtype=np.float32), res


def kernel(**inputs) -> np.ndarray:
    out, _ = _run(inputs, trace=False)
    return out
